# revision 41
# baseline (speedup 1.0000x reference)
"""Trainium2 Bass kernel for BatchedGeometryComputation (segment_reduce).

Strategy (8 cores, data-parallel over atoms, shard boundaries block-aligned):
  - Per-atom layout: atoms along the free dim, 128 lanes per tile, each lane
    a contiguous 512-atom chain; tiles of 65536 atoms stream through SBUF.
  - Segment sums per block via segmented prefix scans (tensor_tensor_scan
    with op0=mult on a "same-block-as-previous" mask): a forward and a
    backward scan give every atom its full block sum and count with no
    gather/scatter: total = fwd_incl + bwd_incl - x.
  - Cross-lane carries (blocks straddling a lane boundary) are fixed with a
    PE shift-matrix matmul + a prefix-of-mask scan + scalar_tensor_tensor
    over the first FIXW columns only.  Backward carries are derived in the
    forward pass as head-run dot products (accum_out), so the whole kernel
    is a single fused sweep: stage B (outputs) lags stage A by one tile and
    reuses its SBUF-resident pos/fwd tiles.
  - centroid/atom = total * recip(count); rel = pos - centroid; dist = sqrt;
    RBF via per-center ACT Square(scale,bias) then one big Exp(-x).
  - centroids output assembled from rel: centroid[b] = pos[e_b] - rel[e_b]
    at each block's last atom.
"""

import math

import numpy as np

LANES = 128
TF = 512                 # atoms per lane per tile
APT = LANES * TF         # atoms per tile
NCORES = 8
FIXW = 128               # carry-fix window (>= max atoms per block)
RBF_DIM = 16
NSQ_DVE = 0              # rbf channels squared on Pool+DVE instead of ACT

_kernel_cache = {}


def _build(ntiles, centers, widths, ndev=NCORES, stage="full"):
    import concourse.bacc as bacc
    import concourse.mybir as mybir
    import concourse.tile as tile
    from contextlib import ExitStack

    f32 = mybir.dt.float32
    i32 = mybir.dt.int32
    bf16 = mybir.dt.bfloat16
    Alu = mybir.AluOpType
    Act = mybir.ActivationFunctionType

    nc = bacc.Bacc("TRN2", target_bir_lowering=False, debug=False,
                   num_devices=ndev)

    pos_d = nc.dram_tensor("pos", [ntiles, LANES, 3 * TF], f32,
                           kind="ExternalInput")
    ide_d = nc.dram_tensor("ide", [ntiles * APT + 2], i32,
                           kind="ExternalInput")
    shm_d = nc.dram_tensor("shm", [LANES, 4 * LANES], f32,
                           kind="ExternalInput")
    rel_d = nc.dram_tensor("rel", [ntiles, LANES, 3 * TF], f32,
                           kind="ExternalOutput")
    dst_d = nc.dram_tensor("dst", [ntiles, LANES, TF], f32,
                           kind="ExternalOutput")
    rbf_d = nc.dram_tensor("rbf", [ntiles, LANES, RBF_DIM * TF], f32,
                           kind="ExternalOutput")

    # per-center immediates
    rinv = [1.0 / (math.sqrt(2.0) * float(w)) for w in widths]
    cbias = [-float(c) * rinv[j] for j, c in enumerate(centers)]

    with tile.TileContext(nc) as tc, ExitStack() as ctx:
        consts = ctx.enter_context(tc.tile_pool(name="consts", bufs=1))
        iop = ctx.enter_context(tc.tile_pool(name="io", bufs=3))
        idp = ctx.enter_context(tc.tile_pool(name="ids", bufs=2))
        fwp = ctx.enter_context(tc.tile_pool(name="fw", bufs=3))
        bwp = ctx.enter_context(tc.tile_pool(name="bw", bufs=3))
        totp = ctx.enter_context(tc.tile_pool(name="tot", bufs=3))
        outp = ctx.enter_context(tc.tile_pool(name="out", bufs=2))
        rbfp = ctx.enter_context(tc.tile_pool(name="rbf", bufs=2))
        smallp = ctx.enter_context(tc.tile_pool(name="small", bufs=2))
        dp = ctx.enter_context(tc.tile_pool(name="dp", bufs=5))
        psp = ctx.enter_context(tc.tile_pool(name="ps", bufs=2, space="PSUM"))
        psbp = ctx.enter_context(tc.tile_pool(name="psb", bufs=3,
                                              space="PSUM"))

        shm = consts.tile([LANES, 4 * LANES], f32)
        nc.sync.dma_start(shm[:, :], shm_d[:, :])
        ones = consts.tile([LANES, TF], f32)
        nc.vector.memset(ones[:, :], 1.0)
        zw = consts.tile([LANES, FIXW], f32)
        nc.vector.memset(zw[:, :], 0.0)
        mball = consts.tile([LANES, (TF + 1) * 2], bf16)
        pmall = consts.tile([LANES, FIXW * 2], f32)
        bvals = consts.tile([LANES, RBF_DIM], f32)
        for j in range(RBF_DIM):
            nc.vector.memset(bvals[:, j:j + 1], cbias[j])

        def eq(out, a, b):
            nc.vector.tensor_tensor(out, a, b, Alu.is_equal)

        def scan(out, d0, d1, init):
            nc.vector.tensor_tensor_scan(out, d0, d1, init, Alu.mult, Alu.add)

        pos_t = {}
        fw_t = {}
        c4_t = {}
        c4b_t = {}
        d2_t = {}
        d_t = {}

        def stage_a(t):
            base = t * APT
            pos = iop.tile([LANES, 3 * TF], f32, tag="pos")
            nc.scalar.dma_start(pos[:, :], pos_d[t])
            pos_t[t] = pos
            ids = idp.tile([LANES, TF], i32, tag="ids")
            nc.scalar.dma_start(
                ids[:, :],
                ide_d[base + 1: base + 1 + APT].rearrange("(p q) -> p q",
                                                          q=TF))
            pcol = smallp.tile([LANES, 1], i32, tag="pcol")
            nc.scalar.dma_start(
                pcol[:, :],
                ide_d[base: base + APT].rearrange("(p q) -> p q",
                                                  q=TF)[:, 0:1])
            ncol = smallp.tile([LANES, 1], i32, tag="ncol")
            nc.scalar.dma_start(
                ncol[:, :],
                ide_d[base + TF + 1: base + APT + 2: TF].unsqueeze(1))

            mt = mball[:, (t % 2) * (TF + 1): (t % 2 + 1) * (TF + 1)]
            eq(mt[:, 1:TF], ids[:, 1:TF], ids[:, 0:TF - 1])
            eq(mt[:, 0:1], ids[:, 0:1], pcol[:, :])
            eq(mt[:, TF:TF + 1], ncol[:, :], ids[:, TF - 1:TF])

            fw = fwp.tile([LANES, 4 * TF], f32, tag="fw")
            fw_t[t] = fw
            for ch in range(3):
                scan(fw[:, ch * TF:(ch + 1) * TF], mt[:, 0:TF],
                     pos[:, ch:3 * TF:3], 0.0)
            scan(fw[:, 3 * TF:4 * TF], mt[:, 0:TF], ones[:, :], 0.0)

            # forward lane-carry: c4[p] = last4[p-1] (+ prev tile's lane 127)
            c4 = psp.tile([LANES, 4], f32, tag="c4")
            c4_t[t] = c4
            nc.tensor.matmul(c4[:, :], shm[:, 0:128],
                             fw[:, TF - 1: 4 * TF: TF],
                             start=True, stop=(t == 0))
            if t > 0:
                nc.tensor.matmul(c4[:, :], shm[:, 128:256],
                                 fw_t[t - 1][:, TF - 1: 4 * TF: TF],
                                 start=False, stop=True)
            pm = pmall[:, (t % 2) * FIXW: (t % 2 + 1) * FIXW]
            scan(pm, mt[:, 0:FIXW], zw[:, :], 1.0)

            # head-run sums (per lane): hs_ch = sum_q pos_ch[q]*pm[q]
            hs = smallp.tile([LANES, 4], f32, tag="hs")
            scr = smallp.tile([LANES, FIXW], f32, tag="scr")
            for ch in range(3):
                nc.vector.scalar_tensor_tensor(
                    scr[:, :], pos[:, ch: 3 * FIXW: 3], 1.0, pm,
                    Alu.mult, Alu.mult, accum_out=hs[:, ch:ch + 1])
            nc.vector.scalar_tensor_tensor(
                scr[:, :], ones[:, 0:FIXW], 1.0, pm,
                Alu.mult, Alu.mult, accum_out=hs[:, 3:4])
            # backward lane-carry: c4b[p] = headsum[p+1] (+ next tile lane 0)
            c4b = psbp.tile([LANES, 4], f32, tag="c4b")
            c4b_t[t] = c4b
            nc.tensor.matmul(c4b[:, :], shm[:, 256:384], hs[:, :],
                             start=True, stop=(t == ntiles - 1))
            if t > 0:
                nc.tensor.matmul(c4b_t[t - 1][:, :], shm[:, 384:512],
                                 hs[:, :], start=False, stop=True)

        def stage_b(t):
            pos = pos_t.pop(t)
            fw = fw_t.pop(t)
            c4 = c4_t.pop(t)
            c4b = c4b_t.pop(t)
            mt = mball[:, (t % 2) * (TF + 1): (t % 2 + 1) * (TF + 1)]
            pm = pmall[:, (t % 2) * FIXW: (t % 2 + 1) * FIXW]

            for ch in range(4):
                nc.vector.scalar_tensor_tensor(
                    fw[:, ch * TF: ch * TF + FIXW], pm,
                    c4[:, ch:ch + 1], fw[:, ch * TF: ch * TF + FIXW],
                    Alu.mult, Alu.add)

            bw = bwp.tile([LANES, 4 * TF], f32, tag="bw")
            mrev = mt[:, TF:0:-1]
            for ch in range(3):
                if ch == 0:
                    o = bw[:, TF - 1::-1]
                else:
                    o = bw[:, ch * TF + TF - 1: ch * TF - 1: -1]
                scan(o, mrev, pos[:, 3 * (TF - 1) + ch::-3], 0.0)
            scan(bw[:, 3 * TF + TF - 1: 3 * TF - 1: -1], mrev, ones[:, :],
                 0.0)
            sm = smallp.tile([LANES, FIXW], f32, tag="sm")
            scan(sm[:, :], mt[:, TF: TF - FIXW: -1], zw[:, :], 1.0)
            for ch in range(4):
                tail = bw[:, ch * TF + TF - 1: ch * TF + TF - 1 - FIXW: -1]
                nc.vector.scalar_tensor_tensor(
                    tail, sm[:, :], c4b[:, ch:ch + 1], tail,
                    Alu.mult, Alu.add)

            # count pipeline decoupled from xyz sums: recip runs while Pool
            # adds the xyz channels
            tcnt = smallp.tile([LANES, TF], f32, tag="tcnt")
            nc.vector.scalar_tensor_tensor(
                tcnt[:, :], fw[:, 3 * TF:4 * TF], -1.0,
                bw[:, 3 * TF:4 * TF], Alu.add, Alu.add)
            rc = smallp.tile([LANES, TF], f32, tag="rc")
            nc.vector.reciprocal(rc[:, :], tcnt[:, :])

            tot = totp.tile([LANES, 3 * TF], f32, tag="tot")
            tot3cm = tot[:, :].rearrange("p (c q) -> p c q", c=3)
            pos3 = pos[:, :].rearrange("p (q c) -> p c q", c=3)
            nc.gpsimd.tensor_tensor(tot[:, :], fw[:, 0:3 * TF],
                                    bw[:, 0:3 * TF], Alu.add)
            nc.gpsimd.tensor_tensor(tot3cm, tot3cm, pos3, Alu.subtract)
            for ch in range(3):
                nc.vector.tensor_tensor(tot[:, ch * TF:(ch + 1) * TF],
                                        tot[:, ch * TF:(ch + 1) * TF],
                                        rc[:, :], Alu.mult)
            rel = outp.tile([LANES, 3 * TF], f32, tag="rel")
            rel3 = rel[:, :].rearrange("p (q c) -> p c q", c=3)
            nc.vector.tensor_tensor(rel3, pos3, tot3cm, Alu.subtract)
            nc.sync.dma_start(rel_d[t], rel[:, :])

            nc.gpsimd.tensor_tensor(tot3cm, rel3, rel3, Alu.mult)
            d2 = dp.tile([LANES, TF], f32, tag="d2")
            nc.vector.tensor_tensor(d2[:, :], tot[:, 0:TF],
                                    tot[:, TF:2 * TF], Alu.add)
            nc.vector.tensor_tensor(d2[:, :], d2[:, :],
                                    tot[:, 2 * TF:3 * TF], Alu.add)
            d2_t[t] = d2

        def stage_sqrt(t):
            d = dp.tile([LANES, TF], f32, tag="d")
            nc.scalar.activation(d[:, :], d2_t.pop(t)[:, :], Act.Sqrt)
            nc.sync.dma_start(dst_d[t], d[:, :])
            d_t[t] = d

        def stage_rbf(t):
            d = d_t.pop(t)
            rb = rbfp.tile([LANES, RBF_DIM * TF], f32, tag="rb")
            for j in range(RBF_DIM):
                nc.scalar.activation(rb[:, j: RBF_DIM * TF: RBF_DIM],
                                     d[:, :], Act.Square,
                                     bias=bvals[:, j:j + 1], scale=rinv[j])
            rb2 = rbfp.tile([LANES, RBF_DIM * TF], f32, tag="rb")
            nc.scalar.activation(rb2[:, :], rb[:, :], Act.Exp, scale=-1.0)
            nc.sync.dma_start(rbf_d[t], rb2[:, :])

        pend = []

        def flush_pair():
            # sqrt(a), sqrt(b) share one table-set load; Square is present
            # in every set so only Exp forces the second load.
            for u in pend:
                stage_sqrt(u)
            for u in pend:
                stage_rbf(u)
            pend.clear()

        for t in range(ntiles + 1):
            if t < ntiles:
                stage_a(t)
            if t >= 1 and stage != "A":
                stage_b(t - 1)
                pend.append(t - 1)
                if len(pend) == 2:
                    flush_pair()
        if stage != "A":
            flush_pair()

    nc.compile()
    return nc


def _get_kernel(ntiles, centers, widths):
    key = (ntiles, tuple(np.asarray(centers).tolist()),
           tuple(np.asarray(widths).tolist()))
    if key not in _kernel_cache:
        _kernel_cache[key] = _build(ntiles, np.asarray(centers, np.float64),
                                    np.asarray(widths, np.float64))
    return _kernel_cache[key]


def _shift_mats():
    shm = np.zeros((LANES, 4 * LANES), np.float32)
    shm[:, 0:128] = np.eye(LANES, k=1)           # carry down: c[m]=last[m-1]
    shm[127, 128 + 0] = 1.0                      # pick prev-tile lane127 -> 0
    shm[:, 256:384] = np.eye(LANES, k=-1)        # carry up: c[m]=head[m+1]
    shm[0, 384 + 127] = 1.0                      # pick next-tile lane0 -> 127
    return shm


def kernel(atom_positions, block_id, n_blocks, centers, widths):
    from concourse import bass_utils

    pos = np.ascontiguousarray(np.asarray(atom_positions, np.float32))
    ids = np.ascontiguousarray(np.asarray(block_id, np.int32))
    nb = int(n_blocks)
    centers = np.asarray(centers, np.float32)
    widths = np.asarray(widths, np.float32)
    n = pos.shape[0]

    # block boundaries & block-aligned shard splits (index metadata)
    cum = np.searchsorted(ids, np.arange(nb + 1)).astype(np.int64)
    counts_max = int(np.diff(cum).max())
    assert counts_max <= FIXW, f"block too large for FIXW: {counts_max}"
    targets = (np.arange(1, NCORES) * n) // NCORES
    bks = np.searchsorted(cum, targets)
    S = np.concatenate([[0], cum[bks], [n]])
    ak_max = int(np.max(np.diff(S)))
    ntiles = max(1, -(-ak_max // APT))
    apad = ntiles * APT

    nc = _get_kernel(ntiles, centers, widths)
    shm = _shift_mats()

    in_maps = []
    for k in range(NCORES):
        s, e = int(S[k]), int(S[k + 1])
        a = e - s
        p = np.zeros((apad, 3), np.float32)
        p[:a] = pos[s:e]
        ide = np.full((apad + 2,), -1, np.int32)
        ide[0] = -3
        ide[-1] = -2
        ide[1:a + 1] = ids[s:e]
        in_maps.append({
            "pos": p.reshape(ntiles, LANES, 3 * TF),
            "ide": ide,
            "shm": shm,
        })

    res = bass_utils.run_bass_kernel_spmd(nc, in_maps,
                                          core_ids=list(range(NCORES)))

    rel = np.empty((n, 3), np.float32)
    dist = np.empty((n,), np.float32)
    rbf = np.empty((n, RBF_DIM), np.float32)
    for k in range(NCORES):
        s, e = int(S[k]), int(S[k + 1])
        a = e - s
        r = res.results[k]
        rel[s:e] = r["rel"].reshape(apad, 3)[:a]
        dist[s:e] = r["dst"].reshape(apad)[:a]
        rbf[s:e] = r["rbf"].reshape(apad, RBF_DIM)[:a]

    # centroids: at each block's last atom e_b, centroid = pos - rel
    centroids = np.zeros((nb, 3), np.float32)
    nonempty = cum[1:] > cum[:-1]
    eidx = cum[1:][nonempty] - 1
    centroids[nonempty] = pos[eidx] - rel[eidx]
    return centroids, rel, dist, rbf


# revision 43
# speedup vs baseline: 1.1146x; 1.1146x over previous
"""Trainium2 Bass kernel for BatchedGeometryComputation (segment_reduce).

Strategy (8 cores, data-parallel over atoms, shard boundaries block-aligned):
  - Per-atom layout: atoms along the free dim, 128 lanes per tile, each lane
    a contiguous 512-atom chain; tiles of 65536 atoms stream through SBUF.
  - Segment sums per block via segmented prefix scans (tensor_tensor_scan
    with op0=mult on a "same-block-as-previous" mask): a forward and a
    backward scan give every atom its full block sum and count with no
    gather/scatter: total = fwd_incl + bwd_incl - x.
  - Cross-lane carries (blocks straddling a lane boundary) are fixed with a
    PE shift-matrix matmul + a prefix-of-mask scan + scalar_tensor_tensor
    over the first FIXW columns only.  Backward carries are derived in the
    forward pass as head-run dot products (accum_out), so the whole kernel
    is a single fused sweep: stage B (outputs) lags stage A by one tile and
    reuses its SBUF-resident pos/fwd tiles.
  - centroid/atom = total * recip(count); rel = pos - centroid; dist = sqrt;
    RBF via per-center ACT Square(scale,bias) then one big Exp(-x).
  - centroids output assembled from rel: centroid[b] = pos[e_b] - rel[e_b]
    at each block's last atom.
"""

import math

import numpy as np

LANES = 128
TF = 512                 # atoms per lane per tile
APT = LANES * TF         # atoms per tile
NCORES = 8
FIXW = 128               # carry-fix window (>= max atoms per block)
RBF_DIM = 16
NSQ_DVE = 0              # rbf channels squared on Pool+DVE instead of ACT

_kernel_cache = {}


def _build(ntiles, centers, widths, ndev=NCORES, stage="full"):
    import concourse.bacc as bacc
    import concourse.mybir as mybir
    import concourse.tile as tile
    from contextlib import ExitStack

    f32 = mybir.dt.float32
    i32 = mybir.dt.int32
    bf16 = mybir.dt.bfloat16
    Alu = mybir.AluOpType
    Act = mybir.ActivationFunctionType

    nc = bacc.Bacc("TRN2", target_bir_lowering=False, debug=False,
                   num_devices=ndev)

    pos_d = nc.dram_tensor("pos", [ntiles, LANES, 3 * TF], f32,
                           kind="ExternalInput")
    ide_d = nc.dram_tensor("ide", [ntiles * APT + 2], i32,
                           kind="ExternalInput")
    shm_d = nc.dram_tensor("shm", [LANES, 4 * LANES], f32,
                           kind="ExternalInput")
    rel_d = nc.dram_tensor("rel", [ntiles, LANES, 3 * TF], f32,
                           kind="ExternalOutput")
    dst_d = nc.dram_tensor("dst", [ntiles, LANES, TF], f32,
                           kind="ExternalOutput")
    rbf_d = nc.dram_tensor("rbf", [ntiles, LANES, RBF_DIM * TF], f32,
                           kind="ExternalOutput")

    # per-center immediates
    rinv = [1.0 / (math.sqrt(2.0) * float(w)) for w in widths]
    cbias = [-float(c) * rinv[j] for j, c in enumerate(centers)]

    with tile.TileContext(nc) as tc, ExitStack() as ctx:
        consts = ctx.enter_context(tc.tile_pool(name="consts", bufs=1))
        iop = ctx.enter_context(tc.tile_pool(name="io", bufs=2))
        idp = ctx.enter_context(tc.tile_pool(name="ids", bufs=2))
        fwp = ctx.enter_context(tc.tile_pool(name="fw", bufs=3))
        bwp = ctx.enter_context(tc.tile_pool(name="bw", bufs=2))
        totp = ctx.enter_context(tc.tile_pool(name="tot", bufs=2))
        outp = ctx.enter_context(tc.tile_pool(name="out", bufs=2))
        rbfp = ctx.enter_context(tc.tile_pool(name="rbf", bufs=3))
        smallp = ctx.enter_context(tc.tile_pool(name="small", bufs=2))
        dp = ctx.enter_context(tc.tile_pool(name="dp", bufs=3))
        psp = ctx.enter_context(tc.tile_pool(name="ps", bufs=2, space="PSUM"))
        psbp = ctx.enter_context(tc.tile_pool(name="psb", bufs=3,
                                              space="PSUM"))

        shm = consts.tile([LANES, 4 * LANES], f32)
        nc.sync.dma_start(shm[:, :], shm_d[:, :])
        ones = consts.tile([LANES, TF], f32)
        nc.vector.memset(ones[:, :], 1.0)
        zw = consts.tile([LANES, FIXW], f32)
        nc.vector.memset(zw[:, :], 0.0)
        mball = consts.tile([LANES, (TF + 1) * 2], bf16)
        pmall = consts.tile([LANES, FIXW * 2], f32)
        bvals = consts.tile([LANES, RBF_DIM], f32)
        for j in range(RBF_DIM):
            nc.vector.memset(bvals[:, j:j + 1], cbias[j])

        def eq(out, a, b):
            nc.vector.tensor_tensor(out, a, b, Alu.is_equal)

        def scan(out, d0, d1, init):
            nc.vector.tensor_tensor_scan(out, d0, d1, init, Alu.mult, Alu.add)

        pos_t = {}
        fw_t = {}
        c4_t = {}
        c4b_t = {}
        d2_t = {}
        d_t = {}

        def stage_a(t):
            base = t * APT
            pos = iop.tile([LANES, 3 * TF], f32, tag="pos")
            nc.scalar.dma_start(pos[:, :], pos_d[t])
            pos_t[t] = pos
            ids = idp.tile([LANES, TF], i32, tag="ids")
            nc.scalar.dma_start(
                ids[:, :],
                ide_d[base + 1: base + 1 + APT].rearrange("(p q) -> p q",
                                                          q=TF))
            pcol = smallp.tile([LANES, 1], i32, tag="pcol")
            nc.scalar.dma_start(
                pcol[:, :],
                ide_d[base: base + APT].rearrange("(p q) -> p q",
                                                  q=TF)[:, 0:1])
            ncol = smallp.tile([LANES, 1], i32, tag="ncol")
            nc.scalar.dma_start(
                ncol[:, :],
                ide_d[base + TF + 1: base + APT + 2: TF].unsqueeze(1))

            mt = mball[:, (t % 2) * (TF + 1): (t % 2 + 1) * (TF + 1)]
            eq(mt[:, 1:TF], ids[:, 1:TF], ids[:, 0:TF - 1])
            eq(mt[:, 0:1], ids[:, 0:1], pcol[:, :])
            eq(mt[:, TF:TF + 1], ncol[:, :], ids[:, TF - 1:TF])

            fw = fwp.tile([LANES, 4 * TF], f32, tag="fw")
            fw_t[t] = fw
            for ch in range(3):
                scan(fw[:, ch * TF:(ch + 1) * TF], mt[:, 0:TF],
                     pos[:, ch:3 * TF:3], 0.0)
            scan(fw[:, 3 * TF:4 * TF], mt[:, 0:TF], ones[:, :], 0.0)

            # forward lane-carry: c4[p] = last4[p-1] (+ prev tile's lane 127)
            c4 = psp.tile([LANES, 4], f32, tag="c4")
            c4_t[t] = c4
            nc.tensor.matmul(c4[:, :], shm[:, 0:128],
                             fw[:, TF - 1: 4 * TF: TF],
                             start=True, stop=(t == 0))
            if t > 0:
                nc.tensor.matmul(c4[:, :], shm[:, 128:256],
                                 fw_t[t - 1][:, TF - 1: 4 * TF: TF],
                                 start=False, stop=True)
            pm = pmall[:, (t % 2) * FIXW: (t % 2 + 1) * FIXW]
            scan(pm, mt[:, 0:FIXW], zw[:, :], 1.0)

            # head-run sums (per lane): hs_ch = sum_q pos_ch[q]*pm[q]
            hs = smallp.tile([LANES, 4], f32, tag="hs")
            scr = smallp.tile([LANES, FIXW], f32, tag="scr")
            for ch in range(3):
                nc.vector.scalar_tensor_tensor(
                    scr[:, :], pos[:, ch: 3 * FIXW: 3], 1.0, pm,
                    Alu.mult, Alu.mult, accum_out=hs[:, ch:ch + 1])
            nc.vector.scalar_tensor_tensor(
                scr[:, :], ones[:, 0:FIXW], 1.0, pm,
                Alu.mult, Alu.mult, accum_out=hs[:, 3:4])
            # backward lane-carry: c4b[p] = headsum[p+1] (+ next tile lane 0)
            c4b = psbp.tile([LANES, 4], f32, tag="c4b")
            c4b_t[t] = c4b
            nc.tensor.matmul(c4b[:, :], shm[:, 256:384], hs[:, :],
                             start=True, stop=(t == ntiles - 1))
            if t > 0:
                nc.tensor.matmul(c4b_t[t - 1][:, :], shm[:, 384:512],
                                 hs[:, :], start=False, stop=True)

        def stage_b(t):
            pos = pos_t.pop(t)
            fw = fw_t.pop(t)
            c4 = c4_t.pop(t)
            c4b = c4b_t.pop(t)
            mt = mball[:, (t % 2) * (TF + 1): (t % 2 + 1) * (TF + 1)]
            pm = pmall[:, (t % 2) * FIXW: (t % 2 + 1) * FIXW]

            for ch in range(4):
                nc.vector.scalar_tensor_tensor(
                    fw[:, ch * TF: ch * TF + FIXW], pm,
                    c4[:, ch:ch + 1], fw[:, ch * TF: ch * TF + FIXW],
                    Alu.mult, Alu.add)

            bw = bwp.tile([LANES, 4 * TF], f32, tag="bw")
            mrev = mt[:, TF:0:-1]
            for ch in range(3):
                if ch == 0:
                    o = bw[:, TF - 1::-1]
                else:
                    o = bw[:, ch * TF + TF - 1: ch * TF - 1: -1]
                scan(o, mrev, pos[:, 3 * (TF - 1) + ch::-3], 0.0)
            scan(bw[:, 3 * TF + TF - 1: 3 * TF - 1: -1], mrev, ones[:, :],
                 0.0)
            sm = smallp.tile([LANES, FIXW], f32, tag="sm")
            scan(sm[:, :], mt[:, TF: TF - FIXW: -1], zw[:, :], 1.0)
            for ch in range(4):
                tail = bw[:, ch * TF + TF - 1: ch * TF + TF - 1 - FIXW: -1]
                nc.vector.scalar_tensor_tensor(
                    tail, sm[:, :], c4b[:, ch:ch + 1], tail,
                    Alu.mult, Alu.add)

            # count pipeline decoupled from xyz sums: recip runs while Pool
            # adds the xyz channels
            tcnt = smallp.tile([LANES, TF], f32, tag="tcnt")
            nc.vector.scalar_tensor_tensor(
                tcnt[:, :], fw[:, 3 * TF:4 * TF], -1.0,
                bw[:, 3 * TF:4 * TF], Alu.add, Alu.add)
            rc = smallp.tile([LANES, TF], f32, tag="rc")
            nc.vector.reciprocal(rc[:, :], tcnt[:, :])

            tot = totp.tile([LANES, 3 * TF], f32, tag="tot")
            tot3cm = tot[:, :].rearrange("p (c q) -> p c q", c=3)
            pos3 = pos[:, :].rearrange("p (q c) -> p c q", c=3)
            nc.gpsimd.tensor_tensor(tot[:, :], fw[:, 0:3 * TF],
                                    bw[:, 0:3 * TF], Alu.add)
            nc.gpsimd.tensor_tensor(tot3cm, tot3cm, pos3, Alu.subtract)
            for ch in range(3):
                nc.vector.tensor_tensor(tot[:, ch * TF:(ch + 1) * TF],
                                        tot[:, ch * TF:(ch + 1) * TF],
                                        rc[:, :], Alu.mult)
            rel = outp.tile([LANES, 3 * TF], f32, tag="rel")
            rel3 = rel[:, :].rearrange("p (q c) -> p c q", c=3)
            nc.vector.tensor_tensor(rel3, pos3, tot3cm, Alu.subtract)
            nc.sync.dma_start(rel_d[t], rel[:, :])

            nc.gpsimd.tensor_tensor(tot3cm, rel3, rel3, Alu.mult)
            d2 = dp.tile([LANES, TF], f32, tag="d2")
            nc.vector.tensor_tensor(d2[:, :], tot[:, 0:TF],
                                    tot[:, TF:2 * TF], Alu.add)
            nc.vector.tensor_tensor(d2[:, :], d2[:, :],
                                    tot[:, 2 * TF:3 * TF], Alu.add)
            d2_t[t] = d2

        def stage_sqrt(t):
            d = dp.tile([LANES, TF], f32, tag="d")
            nc.scalar.activation(d[:, :], d2_t.pop(t)[:, :], Act.Sqrt)
            nc.sync.dma_start(dst_d[t], d[:, :])
            d_t[t] = d

        def stage_rbf(t):
            d = d_t.pop(t)
            rb = rbfp.tile([LANES, RBF_DIM * TF], f32, tag="rb")
            for j in range(RBF_DIM):
                nc.scalar.activation(rb[:, j: RBF_DIM * TF: RBF_DIM],
                                     d[:, :], Act.Square,
                                     bias=bvals[:, j:j + 1], scale=rinv[j])
            rb2 = rbfp.tile([LANES, RBF_DIM * TF], f32, tag="rb")
            nc.scalar.activation(rb2[:, :], rb[:, :], Act.Exp, scale=-1.0)
            nc.sync.dma_start(rbf_d[t], rb2[:, :])

        pend = []

        def flush_pair():
            # sqrt(a), sqrt(b) share one table-set load; Square is present
            # in every set so only Exp forces the second load.
            for u in pend:
                stage_sqrt(u)
            for u in pend:
                stage_rbf(u)
            pend.clear()

        for t in range(ntiles + 1):
            if t < ntiles:
                stage_a(t)
            if t >= 1 and stage != "A":
                stage_b(t - 1)
                pend.append(t - 1)
                if len(pend) == 2:
                    flush_pair()
        if stage != "A":
            flush_pair()

    nc.compile()
    return nc


def _get_kernel(ntiles, centers, widths):
    key = (ntiles, tuple(np.asarray(centers).tolist()),
           tuple(np.asarray(widths).tolist()))
    if key not in _kernel_cache:
        _kernel_cache[key] = _build(ntiles, np.asarray(centers, np.float64),
                                    np.asarray(widths, np.float64))
    return _kernel_cache[key]


def _shift_mats():
    shm = np.zeros((LANES, 4 * LANES), np.float32)
    shm[:, 0:128] = np.eye(LANES, k=1)           # carry down: c[m]=last[m-1]
    shm[127, 128 + 0] = 1.0                      # pick prev-tile lane127 -> 0
    shm[:, 256:384] = np.eye(LANES, k=-1)        # carry up: c[m]=head[m+1]
    shm[0, 384 + 127] = 1.0                      # pick next-tile lane0 -> 127
    return shm


def kernel(atom_positions, block_id, n_blocks, centers, widths):
    from concourse import bass_utils

    pos = np.ascontiguousarray(np.asarray(atom_positions, np.float32))
    ids = np.ascontiguousarray(np.asarray(block_id, np.int32))
    nb = int(n_blocks)
    centers = np.asarray(centers, np.float32)
    widths = np.asarray(widths, np.float32)
    n = pos.shape[0]

    # block boundaries & block-aligned shard splits (index metadata)
    cum = np.searchsorted(ids, np.arange(nb + 1)).astype(np.int64)
    counts_max = int(np.diff(cum).max())
    assert counts_max <= FIXW, f"block too large for FIXW: {counts_max}"
    targets = (np.arange(1, NCORES) * n) // NCORES
    bks = np.searchsorted(cum, targets)
    S = np.concatenate([[0], cum[bks], [n]])
    ak_max = int(np.max(np.diff(S)))
    ntiles = max(1, -(-ak_max // APT))
    apad = ntiles * APT

    nc = _get_kernel(ntiles, centers, widths)
    shm = _shift_mats()

    in_maps = []
    for k in range(NCORES):
        s, e = int(S[k]), int(S[k + 1])
        a = e - s
        p = np.zeros((apad, 3), np.float32)
        p[:a] = pos[s:e]
        ide = np.full((apad + 2,), -1, np.int32)
        ide[0] = -3
        ide[-1] = -2
        ide[1:a + 1] = ids[s:e]
        in_maps.append({
            "pos": p.reshape(ntiles, LANES, 3 * TF),
            "ide": ide,
            "shm": shm,
        })

    res = bass_utils.run_bass_kernel_spmd(nc, in_maps,
                                          core_ids=list(range(NCORES)))

    rel = np.empty((n, 3), np.float32)
    dist = np.empty((n,), np.float32)
    rbf = np.empty((n, RBF_DIM), np.float32)
    for k in range(NCORES):
        s, e = int(S[k]), int(S[k + 1])
        a = e - s
        r = res.results[k]
        rel[s:e] = r["rel"].reshape(apad, 3)[:a]
        dist[s:e] = r["dst"].reshape(apad)[:a]
        rbf[s:e] = r["rbf"].reshape(apad, RBF_DIM)[:a]

    # centroids: at each block's last atom e_b, centroid = pos - rel
    centroids = np.zeros((nb, 3), np.float32)
    nonempty = cum[1:] > cum[:-1]
    eidx = cum[1:][nonempty] - 1
    centroids[nonempty] = pos[eidx] - rel[eidx]
    return centroids, rel, dist, rbf


# revision 46
# speedup vs baseline: 1.1317x; 1.0153x over previous
"""Trainium2 Bass kernel for BatchedGeometryComputation (segment_reduce).

Strategy (8 cores, data-parallel over atoms, shard boundaries block-aligned):
  - Per-atom layout: atoms along the free dim, 128 lanes per tile, each lane
    a contiguous 512-atom chain; tiles of 65536 atoms stream through SBUF.
  - Segment sums per block via segmented prefix scans (tensor_tensor_scan
    with op0=mult on a "same-block-as-previous" mask): a forward and a
    backward scan give every atom its full block sum and count with no
    gather/scatter: total = fwd_incl + bwd_incl - x.
  - Cross-lane carries (blocks straddling a lane boundary) are fixed with a
    PE shift-matrix matmul + a prefix-of-mask scan + scalar_tensor_tensor
    over the first FIXW columns only.  Backward carries are derived in the
    forward pass as head-run dot products (accum_out), so the whole kernel
    is a single fused sweep: stage B (outputs) lags stage A by one tile and
    reuses its SBUF-resident pos/fwd tiles.
  - centroid/atom = total * recip(count); rel = pos - centroid; dist = sqrt;
    RBF via per-center ACT Square(scale,bias) then one big Exp(-x).
  - centroids output assembled from rel: centroid[b] = pos[e_b] - rel[e_b]
    at each block's last atom.
"""

import math

import numpy as np

LANES = 128
TF = 512                 # atoms per lane per tile
APT = LANES * TF         # atoms per tile
NCORES = 8
FIXW = 128               # carry-fix window (>= max atoms per block)
RBF_DIM = 16
NSQ_DVE = 0              # rbf channels squared on Pool+DVE instead of ACT

_kernel_cache = {}


def _build(ntiles, centers, widths, ndev=NCORES, stage="full"):
    import concourse.bacc as bacc
    import concourse.mybir as mybir
    import concourse.tile as tile
    from contextlib import ExitStack

    f32 = mybir.dt.float32
    i32 = mybir.dt.int32
    bf16 = mybir.dt.bfloat16
    Alu = mybir.AluOpType
    Act = mybir.ActivationFunctionType

    nc = bacc.Bacc("TRN2", target_bir_lowering=False, debug=False,
                   num_devices=ndev)

    pos_d = nc.dram_tensor("pos", [ntiles, LANES, 3 * TF], f32,
                           kind="ExternalInput")
    ide_d = nc.dram_tensor("ide", [ntiles * APT + 2], i32,
                           kind="ExternalInput")
    shm_d = nc.dram_tensor("shm", [LANES, 4 * LANES], f32,
                           kind="ExternalInput")
    rel_d = nc.dram_tensor("rel", [ntiles, LANES, 3 * TF], f32,
                           kind="ExternalOutput")
    dst_d = nc.dram_tensor("dst", [ntiles, LANES, TF], f32,
                           kind="ExternalOutput")
    rbf_d = nc.dram_tensor("rbf", [ntiles, LANES, RBF_DIM * TF], f32,
                           kind="ExternalOutput")

    # per-center immediates
    rinv = [1.0 / (math.sqrt(2.0) * float(w)) for w in widths]
    cbias = [-float(c) * rinv[j] for j, c in enumerate(centers)]

    with tile.TileContext(nc) as tc, ExitStack() as ctx:
        consts = ctx.enter_context(tc.tile_pool(name="consts", bufs=1))
        iop = ctx.enter_context(tc.tile_pool(name="io", bufs=2))
        idp = ctx.enter_context(tc.tile_pool(name="ids", bufs=2))
        fwp = ctx.enter_context(tc.tile_pool(name="fw", bufs=3))
        bwp = ctx.enter_context(tc.tile_pool(name="bw", bufs=2))
        totp = ctx.enter_context(tc.tile_pool(name="tot", bufs=2))
        outp = ctx.enter_context(tc.tile_pool(name="out", bufs=2))
        rbfp = ctx.enter_context(tc.tile_pool(name="rbf", bufs=3))
        smallp = ctx.enter_context(tc.tile_pool(name="small", bufs=2))
        dp = ctx.enter_context(tc.tile_pool(name="dp", bufs=3))
        psp = ctx.enter_context(tc.tile_pool(name="ps", bufs=2, space="PSUM"))
        psbp = ctx.enter_context(tc.tile_pool(name="psb", bufs=3,
                                              space="PSUM"))

        shm = consts.tile([LANES, 4 * LANES], f32)
        nc.sync.dma_start(shm[:, :], shm_d[:, :])
        ones = consts.tile([LANES, TF], f32)
        nc.vector.memset(ones[:, :], 1.0)
        zw = consts.tile([LANES, FIXW], f32)
        nc.vector.memset(zw[:, :], 0.0)
        mball = consts.tile([LANES, (TF + 1) * 2], bf16)
        pmall = consts.tile([LANES, FIXW * 2], f32)
        bvals = consts.tile([LANES, RBF_DIM], f32)
        for j in range(RBF_DIM):
            nc.vector.memset(bvals[:, j:j + 1], cbias[j])

        def eq(out, a, b):
            nc.vector.tensor_tensor(out, a, b, Alu.is_equal)

        def scan(out, d0, d1, init):
            nc.vector.tensor_tensor_scan(out, d0, d1, init, Alu.mult, Alu.add)

        pos_t = {}
        fw_t = {}
        c4_t = {}
        c4b_t = {}
        d2_t = {}
        d_t = {}

        def stage_a(t):
            base = t * APT
            pos = iop.tile([LANES, 3 * TF], f32, tag="pos")
            nc.scalar.dma_start(pos[:, :], pos_d[t])
            pos_t[t] = pos
            ids = idp.tile([LANES, TF], i32, tag="ids")
            nc.scalar.dma_start(
                ids[:, :],
                ide_d[base + 1: base + 1 + APT].rearrange("(p q) -> p q",
                                                          q=TF))
            pcol = smallp.tile([LANES, 1], i32, tag="pcol")
            nc.scalar.dma_start(
                pcol[:, :],
                ide_d[base: base + APT].rearrange("(p q) -> p q",
                                                  q=TF)[:, 0:1])
            ncol = smallp.tile([LANES, 1], i32, tag="ncol")
            nc.scalar.dma_start(
                ncol[:, :],
                ide_d[base + TF + 1: base + APT + 2: TF].unsqueeze(1))

            mt = mball[:, (t % 2) * (TF + 1): (t % 2 + 1) * (TF + 1)]
            eq(mt[:, 1:TF], ids[:, 1:TF], ids[:, 0:TF - 1])
            eq(mt[:, 0:1], ids[:, 0:1], pcol[:, :])
            eq(mt[:, TF:TF + 1], ncol[:, :], ids[:, TF - 1:TF])

            fw = fwp.tile([LANES, 4 * TF], f32, tag="fw")
            fw_t[t] = fw
            for ch in range(3):
                scan(fw[:, ch * TF:(ch + 1) * TF], mt[:, 0:TF],
                     pos[:, ch:3 * TF:3], 0.0)
            scan(fw[:, 3 * TF:4 * TF], mt[:, 0:TF], ones[:, :], 0.0)

            # forward lane-carry: c4[p] = last4[p-1] (+ prev tile's lane 127)
            c4 = psp.tile([LANES, 4], f32, tag="c4")
            c4_t[t] = c4
            nc.tensor.matmul(c4[:, :], shm[:, 0:128],
                             fw[:, TF - 1: 4 * TF: TF],
                             start=True, stop=(t == 0))
            if t > 0:
                nc.tensor.matmul(c4[:, :], shm[:, 128:256],
                                 fw_t[t - 1][:, TF - 1: 4 * TF: TF],
                                 start=False, stop=True)
            pm = pmall[:, (t % 2) * FIXW: (t % 2 + 1) * FIXW]
            scan(pm, mt[:, 0:FIXW], zw[:, :], 1.0)

            # head-run sums (per lane): hs_ch = sum_q pos_ch[q]*pm[q]
            hs = smallp.tile([LANES, 4], f32, tag="hs")
            scr = smallp.tile([LANES, FIXW], f32, tag="scr")
            for ch in range(3):
                nc.vector.scalar_tensor_tensor(
                    scr[:, :], pos[:, ch: 3 * FIXW: 3], 1.0, pm,
                    Alu.mult, Alu.mult, accum_out=hs[:, ch:ch + 1])
            nc.vector.scalar_tensor_tensor(
                scr[:, :], ones[:, 0:FIXW], 1.0, pm,
                Alu.mult, Alu.mult, accum_out=hs[:, 3:4])
            # backward lane-carry: c4b[p] = headsum[p+1] (+ next tile lane 0)
            c4b = psbp.tile([LANES, 4], f32, tag="c4b")
            c4b_t[t] = c4b
            nc.tensor.matmul(c4b[:, :], shm[:, 256:384], hs[:, :],
                             start=True, stop=(t == ntiles - 1))
            if t > 0:
                nc.tensor.matmul(c4b_t[t - 1][:, :], shm[:, 384:512],
                                 hs[:, :], start=False, stop=True)

        def stage_b(t):
            pos = pos_t.pop(t)
            fw = fw_t.pop(t)
            c4 = c4_t.pop(t)
            c4b = c4b_t.pop(t)
            mt = mball[:, (t % 2) * (TF + 1): (t % 2 + 1) * (TF + 1)]
            pm = pmall[:, (t % 2) * FIXW: (t % 2 + 1) * FIXW]

            for ch in range(4):
                nc.vector.scalar_tensor_tensor(
                    fw[:, ch * TF: ch * TF + FIXW], pm,
                    c4[:, ch:ch + 1], fw[:, ch * TF: ch * TF + FIXW],
                    Alu.mult, Alu.add)

            bw = bwp.tile([LANES, 4 * TF], f32, tag="bw")
            mrev = mt[:, TF:0:-1]
            for ch in range(3):
                if ch == 0:
                    o = bw[:, TF - 1::-1]
                else:
                    o = bw[:, ch * TF + TF - 1: ch * TF - 1: -1]
                scan(o, mrev, pos[:, 3 * (TF - 1) + ch::-3], 0.0)
            scan(bw[:, 3 * TF + TF - 1: 3 * TF - 1: -1], mrev, ones[:, :],
                 0.0)
            sm = smallp.tile([LANES, FIXW], f32, tag="sm")
            scan(sm[:, :], mt[:, TF: TF - FIXW: -1], zw[:, :], 1.0)
            for ch in range(4):
                tail = bw[:, ch * TF + TF - 1: ch * TF + TF - 1 - FIXW: -1]
                nc.vector.scalar_tensor_tensor(
                    tail, sm[:, :], c4b[:, ch:ch + 1], tail,
                    Alu.mult, Alu.add)

            # count pipeline decoupled from xyz sums: recip runs while Pool
            # adds the xyz channels
            tcnt = smallp.tile([LANES, TF], f32, tag="tcnt")
            nc.vector.scalar_tensor_tensor(
                tcnt[:, :], fw[:, 3 * TF:4 * TF], -1.0,
                bw[:, 3 * TF:4 * TF], Alu.add, Alu.add)
            rc = smallp.tile([LANES, TF], f32, tag="rc")
            nc.vector.reciprocal(rc[:, :], tcnt[:, :])

            tot = totp.tile([LANES, 3 * TF], f32, tag="tot")
            tot3cm = tot[:, :].rearrange("p (c q) -> p c q", c=3)
            pos3 = pos[:, :].rearrange("p (q c) -> p c q", c=3)
            nc.gpsimd.tensor_tensor(tot[:, :], fw[:, 0:3 * TF],
                                    bw[:, 0:3 * TF], Alu.add)
            nc.gpsimd.tensor_tensor(tot3cm, tot3cm, pos3, Alu.subtract)
            for ch in range(3):
                nc.vector.tensor_tensor(tot[:, ch * TF:(ch + 1) * TF],
                                        tot[:, ch * TF:(ch + 1) * TF],
                                        rc[:, :], Alu.mult)
            rel = outp.tile([LANES, 3 * TF], f32, tag="rel")
            rel3 = rel[:, :].rearrange("p (q c) -> p c q", c=3)
            nc.vector.tensor_tensor(rel3, pos3, tot3cm, Alu.subtract)
            nc.sync.dma_start(rel_d[t], rel[:, :])

            nc.gpsimd.tensor_tensor(tot3cm, rel3, rel3, Alu.mult)
            d2 = dp.tile([LANES, TF], f32, tag="d2")
            nc.vector.tensor_tensor(d2[:, :], tot[:, 0:TF],
                                    tot[:, TF:2 * TF], Alu.add)
            nc.vector.tensor_tensor(d2[:, :], d2[:, :],
                                    tot[:, 2 * TF:3 * TF], Alu.add)
            d2_t[t] = d2

        def stage_sqrt(t):
            d = dp.tile([LANES, TF], f32, tag="d")
            nc.scalar.activation(d[:, :], d2_t.pop(t)[:, :], Act.Sqrt)
            nc.sync.dma_start(dst_d[t], d[:, :])
            d_t[t] = d

        def stage_rbf(t, split=False):
            d = d_t.pop(t)
            rb = rbfp.tile([LANES, RBF_DIM * TF], f32, tag="rb")
            for j in range(RBF_DIM):
                nc.scalar.activation(rb[:, j: RBF_DIM * TF: RBF_DIM],
                                     d[:, :], Act.Square,
                                     bias=bvals[:, j:j + 1], scale=rinv[j])
            rb2 = rbfp.tile([LANES, RBF_DIM * TF], f32, tag="rb")
            if not split:
                nc.scalar.activation(rb2[:, :], rb[:, :], Act.Exp, scale=-1.0)
                nc.sync.dma_start(rbf_d[t], rb2[:, :])
            else:
                # tail tiles: halve the exp+store so the final 4.2MB DMA
                # overlaps the remaining ACT work instead of running exposed
                h = RBF_DIM * TF // 2
                for k in range(2):
                    nc.scalar.activation(rb2[:, k * h:(k + 1) * h],
                                         rb[:, k * h:(k + 1) * h],
                                         Act.Exp, scale=-1.0)
                    nc.sync.dma_start(rbf_d[t][:, k * h:(k + 1) * h],
                                      rb2[:, k * h:(k + 1) * h])

        pend = []

        def flush_pair(last=False):
            # sqrt(a), sqrt(b) share one table-set load; Square is present
            # in every set so only Exp forces the second load.
            for u in pend:
                stage_sqrt(u)
            for i, u in enumerate(pend):
                stage_rbf(u, split=last and i == len(pend) - 1)
            pend.clear()

        for t in range(ntiles + 1):
            if t < ntiles:
                stage_a(t)
            if t >= 1 and stage != "A":
                stage_b(t - 1)
                pend.append(t - 1)
                if len(pend) == 2:
                    flush_pair(last=(t == ntiles))
        if stage != "A":
            flush_pair(last=True)

    nc.compile()
    return nc


def _get_kernel(ntiles, centers, widths):
    key = (ntiles, tuple(np.asarray(centers).tolist()),
           tuple(np.asarray(widths).tolist()))
    if key not in _kernel_cache:
        _kernel_cache[key] = _build(ntiles, np.asarray(centers, np.float64),
                                    np.asarray(widths, np.float64))
    return _kernel_cache[key]


def _shift_mats():
    shm = np.zeros((LANES, 4 * LANES), np.float32)
    shm[:, 0:128] = np.eye(LANES, k=1)           # carry down: c[m]=last[m-1]
    shm[127, 128 + 0] = 1.0                      # pick prev-tile lane127 -> 0
    shm[:, 256:384] = np.eye(LANES, k=-1)        # carry up: c[m]=head[m+1]
    shm[0, 384 + 127] = 1.0                      # pick next-tile lane0 -> 127
    return shm


def kernel(atom_positions, block_id, n_blocks, centers, widths):
    from concourse import bass_utils

    pos = np.ascontiguousarray(np.asarray(atom_positions, np.float32))
    ids = np.ascontiguousarray(np.asarray(block_id, np.int32))
    nb = int(n_blocks)
    centers = np.asarray(centers, np.float32)
    widths = np.asarray(widths, np.float32)
    n = pos.shape[0]

    # block boundaries & block-aligned shard splits (index metadata)
    cum = np.searchsorted(ids, np.arange(nb + 1)).astype(np.int64)
    counts_max = int(np.diff(cum).max())
    assert counts_max <= FIXW, f"block too large for FIXW: {counts_max}"
    targets = (np.arange(1, NCORES) * n) // NCORES
    bks = np.searchsorted(cum, targets)
    S = np.concatenate([[0], cum[bks], [n]])
    ak_max = int(np.max(np.diff(S)))
    ntiles = max(1, -(-ak_max // APT))
    apad = ntiles * APT

    nc = _get_kernel(ntiles, centers, widths)
    shm = _shift_mats()

    in_maps = []
    for k in range(NCORES):
        s, e = int(S[k]), int(S[k + 1])
        a = e - s
        p = np.zeros((apad, 3), np.float32)
        p[:a] = pos[s:e]
        ide = np.full((apad + 2,), -1, np.int32)
        ide[0] = -3
        ide[-1] = -2
        ide[1:a + 1] = ids[s:e]
        in_maps.append({
            "pos": p.reshape(ntiles, LANES, 3 * TF),
            "ide": ide,
            "shm": shm,
        })

    res = bass_utils.run_bass_kernel_spmd(nc, in_maps,
                                          core_ids=list(range(NCORES)))

    rel = np.empty((n, 3), np.float32)
    dist = np.empty((n,), np.float32)
    rbf = np.empty((n, RBF_DIM), np.float32)
    for k in range(NCORES):
        s, e = int(S[k]), int(S[k + 1])
        a = e - s
        r = res.results[k]
        rel[s:e] = r["rel"].reshape(apad, 3)[:a]
        dist[s:e] = r["dst"].reshape(apad)[:a]
        rbf[s:e] = r["rbf"].reshape(apad, RBF_DIM)[:a]

    # centroids: at each block's last atom e_b, centroid = pos - rel
    centroids = np.zeros((nb, 3), np.float32)
    nonempty = cum[1:] > cum[:-1]
    eidx = cum[1:][nonempty] - 1
    centroids[nonempty] = pos[eidx] - rel[eidx]
    return centroids, rel, dist, rbf
